# revision 23
# baseline (speedup 1.0000x reference)
"""Distributed 3-layer GAT + global mean pool + linear head on 8 Trainium2
NeuronCores (Bass/Tile, SPMD via run_bass_kernel_spmd).

v2: gather-bound redesign.
  - 1D node partitioning by destination; per-core table slice of 6272 rows
    (49 blocks x 128; device 7 padded).
  - Table rows in BF16: [h(256) | alsrc(4) | pad] -> 768B gather elements
    (L3: [h(64) | alsrc(1) | pad] -> 256B).
  - dense: table_slice = xT_own @ Waug (bf16 matmuls); own-block h / alsrc /
    aldst kept in SBUF for the self-loop diagonal path; aldst also spilled to
    a local bf16 table for the per-edge ALD gather.
  - AllGather (bf16, Shared output) -> full table.
  - edge: self-loops are NOT in the edge stream. Packed schedule: per
    (group of 2 blocks, src-range bucket) edges run back-to-back, padded to
    the max across cores only at bucket boundaries. Source rows gathered via
    dma_gather on 2 SWDGE queues (lo bucket -> q0, hi -> q1; ALD alternates);
    one-hot scatter matmuls (bf16) against 256-wide group windows into PSUM.
  - finalize per block: denom/messages += self-loop term (all-SBUF data),
    /denom + bias + ELU, transpose into xT (L3: one-hot pool matmul).
Pool partials AllReduced, scaled by 1/cnt, final linear -> [64, 10].
"""
import numpy as np

import concourse.bass as bass
import concourse.bacc as bacc
import concourse.tile as tile
from concourse import mybir
from concourse import bass_utils

EDGE_MODE = "full"   # "gather" | "compute" | "full" (bisection aid)

f32 = mybir.dt.float32
f32r = mybir.dt.float32r
bf16 = mybir.dt.bfloat16
i16 = mybir.dt.int16
npbf16 = mybir.dt.np(mybir.dt.bfloat16)

# problem constants (hardcoded per contract)
N, IN, HID, HEADS, OUT, NG = 50000, 128, 64, 4, 10, 64
E = 600000
P = 128
NCORES = 8
NB = 49                      # blocks per core
NODES_PER = NB * P           # 6272 rows per core
NTOT = NODES_PER * NCORES    # 50176
R4 = 4 * NODES_PER           # 25088 bucket split (rows of cores 0-3)
GROUP_BLOCKS = 2
GW = GROUP_BLOCKS * P        # 256 one-hot window
NGROUPS = (NB + GROUP_BLOCKS - 1) // GROUP_BLOCKS   # 25
HC = HEADS * HID             # 256
EW12 = 384                   # bf16 table row width layers 1/2 (768B)
EW3 = 128                    # bf16 table row width layer 3 (256B)
ALD_W = 128                  # bf16 ald table row width (256B)
TBATCH = 8                   # tiles per inner edge iteration


# ----------------------------------------------------------------------------
# host-side graph preprocessing
# ----------------------------------------------------------------------------

def _assign_nodes(src, dst):
    """node -> row permutation balancing per-(core,group,bucket) edge counts.

    src/dst exclude the appended self-loops. Returns perm[node] = global row.
    """
    degin = np.bincount(dst, minlength=N).astype(np.int64)
    order = np.argsort(-degin, kind="stable")
    core_load = np.zeros(NCORES, dtype=np.int64)
    core_cnt = np.zeros(NCORES, dtype=np.int64)
    node_core = np.empty(N, dtype=np.int64)
    for n in order:
        c = np.argmin(np.where(core_cnt < NODES_PER, core_load,
                               np.iinfo(np.int64).max))
        node_core[n] = c
        core_load[c] += degin[n]
        core_cnt[c] += 1
    lo_mask = node_core[src] < 4
    deg_lo = np.bincount(dst[lo_mask], minlength=N).astype(np.int64)
    deg_hi = degin - deg_lo
    # per core, nodes -> groups balancing (lo,hi) in-degree, then slot
    perm = np.empty(N, dtype=np.int64)
    gcap = np.full(NGROUPS, GW, dtype=np.int64)
    gcap[NGROUPS - 1] = (NB - (NGROUPS - 1) * GROUP_BLOCKS) * P
    for c in range(NCORES):
        nodes = np.where(node_core == c)[0]
        nd = nodes[np.argsort(-(deg_lo[nodes] + deg_hi[nodes]), kind="stable")]
        glo = np.zeros(NGROUPS, dtype=np.int64)
        ghi = np.zeros(NGROUPS, dtype=np.int64)
        gcnt = np.zeros(NGROUPS, dtype=np.int64)
        slot = np.zeros(NGROUPS, dtype=np.int64)
        for n in nd:
            g = np.argmin(np.where(gcnt < gcap, glo + ghi,
                                   np.iinfo(np.int64).max))
            perm[n] = c * NODES_PER + g * GW + slot[g]
            glo[g] += deg_lo[n]
            ghi[g] += deg_hi[n]
            gcnt[g] += 1
            slot[g] += 1
    return perm


def _build_schedule(srcrow, dstrow):
    """Packed shape-uniform SPMD edge schedule (no self-loops in stream)."""
    core = dstrow // NODES_PER
    loc = dstrow % NODES_PER
    grp = loc // GW
    gdl = loc % GW
    bkt = (srcrow >= R4).astype(np.int64)

    # order: core, group, bucket, block-in-group (gdl)
    order = np.lexsort((gdl, bkt, grp, core))
    s_o = srcrow[order]
    g_o = gdl[order]
    key = ((core * NGROUPS + grp) * 2 + bkt)[order]
    ncell = NCORES * NGROUPS * 2
    cuts = np.searchsorted(key, np.arange(ncell + 1))
    cnt = np.diff(cuts).reshape(NCORES, NGROUPS, 2)

    T = np.zeros((NGROUPS, 2), dtype=np.int64)
    for g in range(NGROUPS):
        for k in (0, 1):
            T[g, k] = (cnt[:, g, k].max() + P - 1) // P

    group_info = []
    toff = 0
    for g in range(NGROUPS):
        group_info.append((toff, int(T[g, 0]), int(T[g, 1])))
        toff += int(T[g, 0] + T[g, 1])
    ntiles = toff

    hidx = np.zeros((NCORES, ntiles * P), dtype=np.int16)
    dloc = np.full((NCORES, ntiles * P), -1.0, dtype=np.float32)
    # block sets per (group, tile-in-group): union over cores
    nblocks_g = [min(GROUP_BLOCKS, NB - g * GROUP_BLOCKS)
                 for g in range(NGROUPS)]
    touch = [np.zeros((T[g, 0] + T[g, 1], nblocks_g[g]), dtype=bool)
             for g in range(NGROUPS)]
    for c in range(NCORES):
        for g in range(NGROUPS):
            toff_g, tlo, thi = group_info[g]
            for k in (0, 1):
                cell = (c * NGROUPS + g) * 2 + k
                s = s_o[cuts[cell]:cuts[cell + 1]]
                d = g_o[cuts[cell]:cuts[cell + 1]]
                n = len(s)
                pos = (toff_g + (0 if k == 0 else tlo)) * P
                hidx[c, pos:pos + n] = (s - k * R4).astype(np.int16)
                dloc[c, pos:pos + n] = d.astype(np.float32)
                jt0 = 0 if k == 0 else tlo
                if n:
                    touch[g][jt0 + np.arange(n) // P, d // P] = True

    # per group: ordered op list [(tile_in_group, brel)] + start/stop flags
    ops_per_group = []
    for g in range(NGROUPS):
        ops = []
        tg = int(T[g, 0] + T[g, 1])
        for jt in range(tg):
            for brel in range(nblocks_g[g]):
                if touch[g][jt, brel]:
                    ops.append((jt, brel))
        first = {}
        last = {}
        for i, (jt, brel) in enumerate(ops):
            if brel not in first:
                first[brel] = i
            last[brel] = i
        flag_ops = [(jt, brel, i == first[brel], i == last[brel])
                    for i, (jt, brel) in enumerate(ops)]
        # blocks with no edges anywhere: mark for memset
        empty = [brel for brel in range(nblocks_g[g]) if brel not in first]
        ops_per_group.append((flag_ops, empty))

    def wrap16(a):
        m = a.reshape(-1, 16).T
        return np.tile(m, (8, 1)).copy()

    return {
        "hidx": np.stack([wrap16(hidx[c]) for c in range(NCORES)]),
        "dloc": np.stack([dloc[c].reshape(ntiles, P).T.copy()
                          for c in range(NCORES)]),   # [128, ntiles]
        # per-edge group-local dst id, replicated on all 128 partitions
        "dloct": np.stack([np.tile(dloc[c].astype(npbf16), (P, 1))
                           for c in range(NCORES)]),  # [128, ntiles*128] bf16
        "group_info": group_info,
        "ops_per_group": ops_per_group,
        "nblocks_g": nblocks_g,
        "ntiles": ntiles,
    }


def _augment_weights(W, asrc, adst):
    """[W | W@Asrc | W@Adst] -> [IN_, HCl + 2H] float32."""
    IN_, HCl = W.shape
    H = asrc.shape[0]
    C = HCl // H
    A_s = np.zeros((HCl, H), np.float32)
    A_d = np.zeros((HCl, H), np.float32)
    for hd in range(H):
        A_s[hd * C:(hd + 1) * C, hd] = asrc[hd]
        A_d[hd * C:(hd + 1) * C, hd] = adst[hd]
    return np.concatenate([W, W @ A_s, W @ A_d], axis=1)


# ----------------------------------------------------------------------------
# device program
# ----------------------------------------------------------------------------

def _build_program(sched, time_reps=1,
                   stages=("d1", "e1", "d2", "e2", "d3", "e3", "pool"),
                   sim_single=False):
    nc = bacc.Bacc("TRN2", target_bir_lowering=False, debug=False,
                   enable_asserts=False,
                   num_devices=1 if sim_single else NCORES,
                   num_swdge_queues=2)
    ntiles = sched["ntiles"]
    group_info = sched["group_info"]
    ops_per_group = sched["ops_per_group"]
    nblocks_g = sched["nblocks_g"]
    tgmax = max(tlo + thi for _, tlo, thi in group_info)

    # ---- kernel I/O ----
    t_xT = nc.dram_tensor("xT", [P, NODES_PER], bf16, kind="ExternalInput")
    t_w1 = nc.dram_tensor("W1a", [P, 264], bf16, kind="ExternalInput")
    t_w2 = nc.dram_tensor("W2a", [P, 2, 264], bf16, kind="ExternalInput")
    t_w3 = nc.dram_tensor("W3a", [P, 2, 66], bf16, kind="ExternalInput")
    t_b1 = nc.dram_tensor("BIAS1", [P, HC], f32, kind="ExternalInput")
    t_b2 = nc.dram_tensor("BIAS2", [P, HC], f32, kind="ExternalInput")
    t_b3 = nc.dram_tensor("BIAS3", [P, HID], f32, kind="ExternalInput")
    t_iota = nc.dram_tensor("IOTA4", [P, TBATCH * GW], f32,
                            kind="ExternalInput")
    t_ident = nc.dram_tensor("IDENT", [P, P], f32, kind="ExternalInput")
    t_hidx = nc.dram_tensor("HIDX", [P, ntiles * 8], i16, kind="ExternalInput")
    t_dloc = nc.dram_tensor("DLOC", [P, ntiles], f32, kind="ExternalInput")
    t_dloct = nc.dram_tensor("DLOCT", [P, ntiles * P], bf16,
                             kind="ExternalInput")
    t_iotap = nc.dram_tensor("IOTAP", [P, GW], bf16, kind="ExternalInput")
    t_bat = nc.dram_tensor("BATCH", [P, NB], f32, kind="ExternalInput")
    t_icnt = nc.dram_tensor("INVCNT", [NG, 1], f32, kind="ExternalInput")
    t_linw = nc.dram_tensor("LINW", [P, OUT], f32r, kind="ExternalInput")
    t_linb = nc.dram_tensor("LINB", [NG, OUT], f32, kind="ExternalInput")
    t_out = nc.dram_tensor("out", [NG, OUT], f32, kind="ExternalOutput")
    t_dbg1 = t_dbg2 = None
    if "pool" not in stages:
        t_dbg1 = nc.dram_tensor("dbg1", [P, 264], f32, kind="ExternalOutput")
        t_dbg2 = nc.dram_tensor("dbg2", [P, 2 * P], f32, kind="ExternalOutput")

    layers = [
        dict(ew=EW12, hc=HC, h=HEADS, nchunk=1, wcols=264, rw=260),
        dict(ew=EW12, hc=HC, h=HEADS, nchunk=2, wcols=264, rw=260),
        dict(ew=EW3, hc=HID, h=1, nchunk=2, wcols=66, rw=66),
    ]

    with tile.TileContext(nc) as tc:
        with tc.tile_pool(name="const", bufs=1) as cpool, \
             tc.tile_pool(name="xT", bufs=1) as xpool, \
             tc.tile_pool(name="gat", bufs=2) as gpool, \
             tc.tile_pool(name="work", bufs=4) as wpool, \
             tc.tile_pool(name="fin", bufs=2) as fpool, \
             tc.tile_pool(name="psacc", bufs=4, space="PSUM") as psacc, \
             tc.tile_pool(name="psmisc", bufs=2, space="PSUM") as psmisc, \
             tc.tile_pool(name="pspool", bufs=1, space="PSUM") as pspool, \
             tc.tile_pool(name="dram", bufs=1, space="DRAM") as dpool:

            # ---- consts into SBUF ----
            iota4 = cpool.tile([P, TBATCH, GW], f32)
            nc.sync.dma_start(iota4[:], t_iota.ap().rearrange(
                "p (t i) -> p t i", t=TBATCH))
            iotap = cpool.tile([P, GROUP_BLOCKS, P], bf16)
            nc.sync.dma_start(iotap[:], t_iotap.ap().rearrange(
                "p (t i) -> p t i", t=GROUP_BLOCKS))
            ident = cpool.tile([P, P], f32)
            nc.sync.dma_start(ident[:], t_ident.ap())
            biases = []
            for tb, w in ((t_b1, HC), (t_b2, HC), (t_b3, HID)):
                bt = cpool.tile([P, w], f32, tag=f"bias{len(biases)}",
                                name=f"bias{len(biases)}")
                nc.sync.dma_start(bt[:], tb.ap())
                biases.append(bt)
            w1t = cpool.tile([P, 1, 264], bf16, tag="w1")
            nc.sync.dma_start(w1t[:, 0, :], t_w1.ap())
            w2t = cpool.tile([P, 2, 264], bf16, tag="w2")
            nc.sync.dma_start(w2t[:], t_w2.ap())
            w3t = cpool.tile([P, 2, 66], bf16, tag="w3")
            nc.sync.dma_start(w3t[:], t_w3.ap())
            w_sb = [w1t, w2t, w3t]
            batc = cpool.tile([P, NB], f32)
            nc.sync.dma_start(batc[:], t_bat.ap())
            icnt = cpool.tile([NG, 1], f32)
            nc.sync.dma_start(icnt[:], t_icnt.ap())
            linw = cpool.tile([P, OUT], f32r)
            nc.sync.dma_start(linw[:], t_linw.ap())
            linb = cpool.tile([NG, OUT], f32)
            nc.sync.dma_start(linb[:], t_linb.ap())

            # persistent transposed activations (bf16) + self-loop side data
            xT = xpool.tile([P, 2, NODES_PER], bf16)
            nc.sync.dma_start(xT[:, 0, :], t_xT.ap())
            # own-block dense outputs [h | alsrc | aldst] for the self-loop
            # path and the aldst one-hot matmuls
            hall = xpool.tile([P, NB, 264], bf16)

            # per-layer DRAM scratch
            tslice = [dpool.tile([NODES_PER, EW12], bf16, tag="ts0", name="ts0"),
                      dpool.tile([NODES_PER, EW12], bf16, tag="ts1", name="ts1"),
                      dpool.tile([NODES_PER, EW3], bf16, tag="ts2", name="ts2")]
            tfull = [dpool.tile([NTOT, EW12], bf16, tag="tf0", name="tf0",
                                addr_space="Shared"),
                     dpool.tile([NTOT, EW12], bf16, tag="tf1", name="tf1",
                                addr_space="Shared"),
                     dpool.tile([NTOT, EW3], bf16, tag="tf2", name="tf2",
                                addr_space="Shared")]
            pool_in = dpool.tile([NG, HID], f32, tag="pin")
            pool_out = dpool.tile([NG, HID], f32, tag="pout")

            pool_ps = pspool.tile([NG, HID], f32, space="PSUM")

            def dense_phase(L):
                lay = layers[L]
                hc, h, nchunk, wcols = lay["hc"], lay["h"], lay["nchunk"], lay["wcols"]
                for m in range(NB):
                    ps = psmisc.tile([P, wcols], f32, space="PSUM", tag="ms",
                                     name="dps")
                    for c in range(nchunk):
                        nc.tensor.matmul(
                            ps[:], xT[:, c, m * P:(m + 1) * P], w_sb[L][:, c, :],
                            start=(c == 0), stop=(c == nchunk - 1))
                    nc.vector.tensor_copy(hall[:, m, 0:hc + 2 * h],
                                          ps[:, 0:hc + 2 * h])
                    nc.sync.dma_start(
                        tslice[L][m * P:(m + 1) * P, 0:hc + h],
                        hall[:, m, 0:hc + h])

            def edge_phase(L):
                lay = layers[L]
                ew, hc, h, rw = lay["ew"], lay["hc"], lay["h"], lay["rw"]
                for g in range(NGROUPS):
                    toff, tlo, thi = group_info[g]
                    flag_ops, empty = ops_per_group[g]
                    tg = tlo + thi
                    nbg = nblocks_g[g]
                    G = gpool.tile([P, tgmax, ew], bf16, tag="G")
                    hix = gpool.tile([P, tgmax * 8], i16, tag="hix")
                    dlc = gpool.tile([P, tgmax], f32, tag="dlc")
                    dlt = gpool.tile([P, tgmax, P], bf16, tag="dlt")
                    o16 = toff * 8
                    nc.sync.dma_start(hix[:, 0:tg * 8],
                                      t_hidx.ap()[:, o16:o16 + tg * 8])
                    nc.sync.dma_start(dlc[:, 0:tg], t_dloc.ap()[:, toff:toff + tg])
                    nc.sync.dma_start(
                        dlt[:, 0:tg, :],
                        t_dloct.ap()[:, toff * P:(toff + tg) * P].rearrange(
                            "p (t i) -> p t i", t=tg))
                    if tlo:
                        nc.gpsimd.dma_gather(
                            G[:, 0:tlo, :], tfull[L][0:R4, :],
                            hix[:, 0:tlo * 8], num_idxs=tlo * P,
                            num_idxs_reg=tlo * P, elem_size=ew, elem_step=ew,
                            single_packet=False, queue_num=0)
                    if thi:
                        nc.gpsimd.dma_gather(
                            G[:, tlo:tg, :], tfull[L][R4:NTOT, :],
                            hix[:, tlo * 8:tg * 8], num_idxs=thi * P,
                            num_idxs_reg=thi * P, elem_size=ew, elem_step=ew,
                            single_packet=False, queue_num=1)

                    if EDGE_MODE == "gather":
                        junk = wpool.tile([P, 8], f32, tag="junk")
                        nc.vector.tensor_tensor(
                            junk[:, 0:4], G[:, 0, 0:4], dlt[:, 0, 0:4],
                            mybir.AluOpType.add)
                        continue

                    accs = {brel: psacc.tile([P, rw], f32, space="PSUM",
                                             tag="acc", name=f"acc{brel}")
                            for brel in range(nbg)}
                    for brel in empty:
                        nc.vector.memset(accs[brel][:], 0.0)

                    # ops grouped by tile batch
                    j = 0
                    oi = 0
                    while j < tg:
                        w = min(TBATCH, tg - j)
                        S = wpool.tile([P, TBATCH, GW], bf16, tag="S")
                        nc.vector.tensor_tensor(
                            S[:, 0:w, :],
                            dlc[:, j:j + w].unsqueeze(-1).to_broadcast(
                                [P, w, GW]),
                            iota4[:, 0:w, :], mybir.AluOpType.is_equal)
                        # per-edge aldst via one-hot matmuls against the
                        # SBUF-resident aldsb (no DMA gather)
                        S2 = wpool.tile([P, TBATCH, GROUP_BLOCKS, P], bf16,
                                        tag="S2")
                        for kc in range(nbg):
                            nc.vector.tensor_tensor(
                                S2[:, 0:w, kc, :], dlt[:, j:j + w, :],
                                iotap[:, kc, :].unsqueeze(1).to_broadcast(
                                    [P, w, P]),
                                mybir.AluOpType.is_equal)
                        ald_ps = psmisc.tile([P, TBATCH, HEADS], f32,
                                             space="PSUM", tag="ms",
                                             name="aldps")
                        for q in range(w):
                            for kc in range(nbg):
                                nc.tensor.matmul(
                                    ald_ps[:, q, 0:h], S2[:, q, kc, :],
                                    hall[:, g * GROUP_BLOCKS + kc,
                                         hc + h:hc + 2 * h],
                                    start=(kc == 0), stop=(kc == nbg - 1),
                                    skip_group_check=True)
                        ald_sb = wpool.tile([P, TBATCH, HEADS], bf16,
                                            tag="aldsbt")
                        nc.vector.tensor_copy(ald_sb[:, 0:w, 0:h],
                                              ald_ps[:, 0:w, 0:h])
                        et = wpool.tile([P, TBATCH, h], f32, tag="et")
                        nc.vector.tensor_tensor(
                            et[:, 0:w, :],
                            G[:, j:j + w, hc:hc + h],
                            ald_sb[:, 0:w, 0:h], mybir.AluOpType.add)
                        lr = wpool.tile([P, TBATCH, h], f32, tag="lr")
                        nc.vector.scalar_tensor_tensor(
                            lr[:, 0:w, :], et[:, 0:w, :], 0.2, et[:, 0:w, :],
                            mybir.AluOpType.mult, mybir.AluOpType.max)
                        ex = wpool.tile([P, TBATCH, h], f32, tag="ex")
                        nc.scalar.activation(ex[:, 0:w, :], lr[:, 0:w, :],
                                             mybir.ActivationFunctionType.Exp)
                        exb = wpool.tile([P, TBATCH, h], bf16, tag="exb")
                        nc.vector.tensor_copy(exb[:, 0:w, :], ex[:, 0:w, :])
                        R = wpool.tile([P, TBATCH, rw], bf16, tag="R")
                        nc.vector.tensor_tensor(
                            R[:, 0:w, 0:hc].rearrange(
                                "p t (hh c) -> p t hh c", hh=h),
                            G[:, j:j + w, 0:hc].rearrange(
                                "p t (hh c) -> p t hh c", hh=h),
                            exb[:, 0:w, :].unsqueeze(-1).to_broadcast(
                                [P, w, h, HID]),
                            mybir.AluOpType.mult)
                        # vector (not scalar.activation Copy): keeps the
                        # Activation engine on Exp only — no act-table reloads
                        if rw == hc + h:
                            nc.vector.tensor_copy(R[:, 0:w, hc:hc + h],
                                                  ex[:, 0:w, :])
                        else:
                            # odd tail (L3): fill cols hc:rw with ex so the
                            # matmul never reads uninitialized SBUF
                            nc.vector.tensor_copy(
                                R[:, 0:w, hc:rw],
                                ex[:, 0:w, 0:1].to_broadcast([P, w, rw - hc]))
                        while oi < len(flag_ops) and flag_ops[oi][0] < j + w:
                            jt, brel, st, sp = flag_ops[oi]
                            nc.tensor.matmul(
                                accs[brel][:], S[:, jt - j, brel * P:(brel + 1) * P],
                                R[:, jt - j, 0:rw],
                                start=st, stop=sp, skip_group_check=True)
                            oi += 1
                        j += w

                    if EDGE_MODE == "compute":
                        for brel in range(nbg):
                            junk2 = wpool.tile([P, 8], f32, tag="junk2")
                            nc.vector.tensor_copy(junk2[:, 0:4],
                                                  accs[brel][:, 0:4])
                        continue

                    for brel in range(nbg):
                        b = g * GROUP_BLOCKS + brel
                        acc = accs[brel]
                        # self-loop attention term (all SBUF)
                        ets = wpool.tile([P, h], f32, tag="ets")
                        nc.vector.tensor_tensor(ets[:], hall[:, b, hc:hc + h],
                                                hall[:, b, hc + h:hc + 2 * h],
                                                mybir.AluOpType.add)
                        lrs = wpool.tile([P, h], f32, tag="lrs")
                        nc.vector.scalar_tensor_tensor(
                            lrs[:], ets[:], 0.2, ets[:],
                            mybir.AluOpType.mult, mybir.AluOpType.max)
                        exs = wpool.tile([P, h], f32, tag="exs")
                        nc.scalar.activation(exs[:], lrs[:],
                                             mybir.ActivationFunctionType.Exp)
                        exsb = wpool.tile([P, h], bf16, tag="exsb")
                        nc.vector.tensor_copy(exsb[:], exs[:])
                        den = wpool.tile([P, h], f32, tag="den")
                        nc.vector.tensor_tensor(den[:], acc[:, hc:hc + h],
                                                exs[:], mybir.AluOpType.add)
                        rec = wpool.tile([P, h], f32, tag="rec")
                        nc.vector.reciprocal(rec[:], den[:])
                        selfc = fpool.tile([P, HC], f32, tag="selfc")
                        nc.vector.tensor_tensor(
                            selfc[:, 0:hc].rearrange("p (hh c) -> p hh c", hh=h),
                            hall[:, b, 0:hc].rearrange("p (hh c) -> p hh c", hh=h),
                            exsb[:].unsqueeze(-1).to_broadcast([P, h, HID]),
                            mybir.AluOpType.mult)
                        num = fpool.tile([P, HC], f32, tag="num")
                        nc.vector.tensor_tensor(num[:, 0:hc], acc[:, 0:hc],
                                                selfc[:, 0:hc],
                                                mybir.AluOpType.add)
                        xb = fpool.tile([P, HC], f32, tag="xb")
                        nc.vector.tensor_tensor(
                            xb[:, 0:hc].rearrange("p (hh c) -> p hh c", hh=h),
                            num[:, 0:hc].rearrange("p (hh c) -> p hh c", hh=h),
                            rec[:].unsqueeze(-1).to_broadcast([P, h, HID]),
                            mybir.AluOpType.mult)
                        nc.vector.tensor_tensor(xb[:, 0:hc], xb[:, 0:hc],
                                                biases[L][:, 0:hc],
                                                mybir.AluOpType.add)
                        # ELU = max(x,0) + exp(min(x,0)) - 1
                        xmin = fpool.tile([P, HC], f32, tag="xmin")
                        nc.vector.tensor_scalar_min(xmin[:, 0:hc], xb[:, 0:hc],
                                                    0.0)
                        em = fpool.tile([P, HC], f32, tag="em")
                        nc.scalar.activation(em[:, 0:hc], xmin[:, 0:hc],
                                             mybir.ActivationFunctionType.Exp)
                        xmax = fpool.tile([P, HC], f32, tag="xmax")
                        nc.vector.tensor_scalar_max(xmax[:, 0:hc], xb[:, 0:hc],
                                                    0.0)
                        if L < 2:
                            x2b = fpool.tile([P, HC], f32, tag="x2b")
                            nc.vector.scalar_tensor_tensor(
                                x2b[:], em[:], -1.0, xmax[:],
                                mybir.AluOpType.add, mybir.AluOpType.add)
                            for cchunk in range(2):
                                pt = psmisc.tile([P, P], f32, space="PSUM",
                                                 tag="ms", name="pt")
                                nc.tensor.transpose(
                                    pt[:], x2b[:, cchunk * P:(cchunk + 1) * P],
                                    ident[:])
                                nc.vector.tensor_copy(
                                    xT[:, cchunk, b * P:(b + 1) * P], pt[:])
                        else:
                            x4 = fpool.tile([P, HID], bf16, tag="x4")
                            nc.vector.scalar_tensor_tensor(
                                x4[:], em[:, 0:HID], -1.0, xmax[:, 0:HID],
                                mybir.AluOpType.add, mybir.AluOpType.add)
                            bsel = fpool.tile([P, NG], bf16, tag="bsel")
                            nc.vector.tensor_tensor(
                                bsel[:],
                                batc[:, b:b + 1].to_broadcast([P, NG]),
                                iota4[:, 0, 0:NG], mybir.AluOpType.is_equal)
                            nc.tensor.matmul(
                                pool_ps[:], bsel[:], x4[:],
                                start=(b == 0), stop=(b == NB - 1),
                                skip_group_check=True)

            reps = max(1, int(time_reps))

            def repeat(fn):
                if reps == 1:
                    fn()
                else:
                    with tc.For_i(0, reps, 1):
                        fn()

            for L in range(3):
                if f"d{L + 1}" in stages:
                    repeat(lambda L=L: dense_phase(L))
                    if sim_single:
                        for k in range(NCORES):
                            nc.sync.dma_start(
                                tfull[L][k * NODES_PER:(k + 1) * NODES_PER, :],
                                tslice[L][:, :])
                    else:
                        nc.gpsimd.collective_compute(
                            "AllGather", mybir.AluOpType.bypass,
                            replica_groups=[list(range(NCORES))],
                            ins=[tslice[L].opt()], outs=[tfull[L].opt()])
                if f"e{L + 1}" in stages:
                    repeat(lambda L=L: edge_phase(L))

            if "pool" not in stages:
                dts = wpool.tile([P, 264], f32, tag="dts")
                nc.vector.memset(dts[:], 0.0)
                nc.sync.dma_start(t_dbg1.ap(), dts[:])
                dxt = wpool.tile([P, 2 * P], f32, tag="dxt")
                nc.vector.tensor_copy(dxt[:, 0:P], xT[:, 0, 0:P])
                nc.vector.tensor_copy(dxt[:, P:2 * P], xT[:, 1, 0:P])
                nc.sync.dma_start(t_dbg2.ap(), dxt[:])
                fin0 = wpool.tile([NG, OUT], f32, tag="finout")
                nc.vector.memset(fin0[:], 0.0)
                nc.sync.dma_start(t_out.ap(), fin0[:])
            else:
                pp = wpool.tile([NG, HID], f32, tag="pp")
                nc.vector.tensor_copy(pp[:], pool_ps[:])
                if sim_single:
                    nc.sync.dma_start(pool_in[:], pp[:])
                    nc.sync.dma_start(pool_out[:], pool_in[:])
                else:
                    nc.sync.dma_start(pool_in[:], pp[:])
                    nc.gpsimd.collective_compute(
                        "AllReduce", mybir.AluOpType.add,
                        replica_groups=[list(range(NCORES))],
                        ins=[pool_in.opt()], outs=[pool_out.opt()])
                pooled = wpool.tile([NG, HID], f32, tag="pooled")
                nc.sync.dma_start(pooled[:], pool_out[:])
                pscal = wpool.tile([NG, HID], f32, tag="pscal")
                nc.scalar.activation(pscal[:], pooled[:],
                                     mybir.ActivationFunctionType.Copy,
                                     scale=icnt[:])
                ptp = psmisc.tile([NG, NG], f32, space="PSUM", tag="ms",
                                  name="ptp")
                nc.tensor.transpose(ptp[:], pscal[:, 0:NG], ident[0:NG, 0:NG])
                zconst = wpool.tile([P, NG], f32, tag="zconst")
                nc.vector.memset(zconst[:], 0.0)
                pT = wpool.tile([P, NG], f32r, tag="pT")
                nc.vector.tensor_copy(pT[:], zconst[:])
                nc.vector.tensor_copy(pT[0:NG, :], ptp[:])
                ops = psmisc.tile([NG, OUT], f32, space="PSUM", tag="ms",
                                  name="ops")
                nc.tensor.matmul(ops[:], pT[:], linw[:], start=True, stop=True)
                fin = wpool.tile([NG, OUT], f32, tag="finout")
                nc.vector.tensor_tensor(fin[:], ops[:], linb[:],
                                        mybir.AluOpType.add)
                nc.sync.dma_start(t_out.ap(), fin[:])

    nc.compile()
    return nc


# ----------------------------------------------------------------------------
# host orchestration
# ----------------------------------------------------------------------------

def _prepare(inputs):
    x = np.asarray(inputs["x"], dtype=np.float32)
    ei = np.asarray(inputs["edge_index"])
    batch = np.asarray(inputs["batch"])
    src = ei[0].astype(np.int64)
    dst = ei[1].astype(np.int64)

    perm = _assign_nodes(src, dst)
    srcrow = perm[src]
    dstrow = perm[dst]
    sched = _build_schedule(srcrow, dstrow)

    row_node = np.full(NTOT, -1, dtype=np.int64)
    row_node[perm] = np.arange(N)

    w1a = _augment_weights(np.asarray(inputs["W1"], np.float32),
                           np.asarray(inputs["asrc1"], np.float32),
                           np.asarray(inputs["adst1"], np.float32))
    w2a = _augment_weights(np.asarray(inputs["W2"], np.float32),
                           np.asarray(inputs["asrc2"], np.float32),
                           np.asarray(inputs["adst2"], np.float32))
    w3a = _augment_weights(np.asarray(inputs["W3"], np.float32),
                           np.asarray(inputs["asrc3"], np.float32),
                           np.asarray(inputs["adst3"], np.float32))

    iota4 = np.tile(np.arange(GW, dtype=np.float32), (P, TBATCH)).reshape(
        P, TBATCH * GW)
    iotap = np.zeros((P, GW), np.float32)
    for kc in range(GROUP_BLOCKS):
        iotap[:, kc * P:(kc + 1) * P] = (kc * P + np.arange(P))[:, None]
    iotap = iotap.astype(npbf16)
    ident = np.eye(P, dtype=np.float32)

    cnts = np.bincount(batch, minlength=NG).astype(np.float32)
    invcnt = (1.0 / np.maximum(cnts, 1.0)).reshape(NG, 1)
    linb = np.tile(np.asarray(inputs["linb"], np.float32), (NG, 1))

    in_maps = []
    for c in range(NCORES):
        rows = row_node[c * NODES_PER:(c + 1) * NODES_PER]
        xT = np.zeros((P, NODES_PER), npbf16)
        valid = rows >= 0
        xT[:, valid] = x[rows[valid]].T.astype(npbf16)
        batc = np.full((NB, P), -1.0, np.float32)
        bflat = batc.reshape(-1)
        bflat[valid] = batch[rows[valid]].astype(np.float32)
        in_maps.append({
            "xT": xT,
            "W1a": w1a.astype(npbf16),
            "W2a": np.stack([w2a[0:P], w2a[P:2 * P]], axis=1).astype(npbf16),
            "W3a": np.stack([w3a[0:P], w3a[P:2 * P]], axis=1).astype(npbf16),
            "BIAS1": np.tile(np.asarray(inputs["b1"], np.float32), (P, 1)),
            "BIAS2": np.tile(np.asarray(inputs["b2"], np.float32), (P, 1)),
            "BIAS3": np.tile(np.asarray(inputs["b3"], np.float32), (P, 1)),
            "IOTA4": iota4,
            "IOTAP": iotap,
            "IDENT": ident,
            "HIDX": sched["hidx"][c],
            "DLOC": sched["dloc"][c],
            "DLOCT": sched["dloct"][c],
            "BATCH": batc.T.copy(),
            "INVCNT": invcnt,
            "LINW": np.concatenate(
                [np.asarray(inputs["linW"], np.float32),
                 np.zeros((P - HID, OUT), np.float32)], axis=0),
            "LINB": linb,
        })
    return sched, in_maps


def kernel(**inputs):
    sched, in_maps = _prepare(inputs)
    nc = _build_program(sched, time_reps=1)
    res = bass_utils.run_bass_kernel_spmd(nc, in_maps, core_ids=list(range(NCORES)))
    return res.results[0]["out"].astype(np.float32)


# revision 24
# speedup vs baseline: 1.3360x; 1.3360x over previous
"""Distributed 3-layer GAT + global mean pool + linear head on 8 Trainium2
NeuronCores (Bass/Tile, SPMD via run_bass_kernel_spmd).

v2: gather-bound redesign.
  - 1D node partitioning by destination; per-core table slice of 6272 rows
    (49 blocks x 128; device 7 padded).
  - Table rows in BF16: [h(256) | alsrc(4) | pad] -> 768B gather elements
    (L3: [h(64) | alsrc(1) | pad] -> 256B).
  - dense: table_slice = xT_own @ Waug (bf16 matmuls); own-block h / alsrc /
    aldst kept in SBUF for the self-loop diagonal path; aldst also spilled to
    a local bf16 table for the per-edge ALD gather.
  - AllGather (bf16, Shared output) -> full table.
  - edge: self-loops are NOT in the edge stream. Packed schedule: per
    (group of 2 blocks, src-range bucket) edges run back-to-back, padded to
    the max across cores only at bucket boundaries. Source rows gathered via
    dma_gather on 2 SWDGE queues (lo bucket -> q0, hi -> q1; ALD alternates);
    one-hot scatter matmuls (bf16) against 256-wide group windows into PSUM.
  - finalize per block: denom/messages += self-loop term (all-SBUF data),
    /denom + bias + ELU, transpose into xT (L3: one-hot pool matmul).
Pool partials AllReduced, scaled by 1/cnt, final linear -> [64, 10].
"""
import numpy as np

import concourse.bass as bass
import concourse.bacc as bacc
import concourse.tile as tile
from concourse import mybir
from concourse import bass_utils

EDGE_MODE = "full"   # "gather" | "compute" | "full" (bisection aid)

f32 = mybir.dt.float32
f32r = mybir.dt.float32r
bf16 = mybir.dt.bfloat16
i16 = mybir.dt.int16
npbf16 = mybir.dt.np(mybir.dt.bfloat16)

# problem constants (hardcoded per contract)
N, IN, HID, HEADS, OUT, NG = 50000, 128, 64, 4, 10, 64
E = 600000
P = 128
NCORES = 8
NB = 49                      # blocks per core
NODES_PER = NB * P           # 6272 rows per core
NTOT = NODES_PER * NCORES    # 50176
R4 = 4 * NODES_PER           # 25088 bucket split (rows of cores 0-3)
GROUP_BLOCKS = 2
GW = GROUP_BLOCKS * P        # 256 one-hot window
NGROUPS = (NB + GROUP_BLOCKS - 1) // GROUP_BLOCKS   # 25
HC = HEADS * HID             # 256
EW12 = 384                   # bf16 table row width layers 1/2 (768B)
EW3 = 128                    # bf16 table row width layer 3 (256B)
ALD_W = 128                  # bf16 ald table row width (256B)
TBATCH = 4                   # tiles per inner edge iteration


# ----------------------------------------------------------------------------
# host-side graph preprocessing
# ----------------------------------------------------------------------------

def _assign_nodes(src, dst):
    """node -> row permutation balancing per-(core,group,bucket) edge counts.

    src/dst exclude the appended self-loops. Returns perm[node] = global row.
    """
    degin = np.bincount(dst, minlength=N).astype(np.int64)
    order = np.argsort(-degin, kind="stable")
    core_load = np.zeros(NCORES, dtype=np.int64)
    core_cnt = np.zeros(NCORES, dtype=np.int64)
    node_core = np.empty(N, dtype=np.int64)
    for n in order:
        c = np.argmin(np.where(core_cnt < NODES_PER, core_load,
                               np.iinfo(np.int64).max))
        node_core[n] = c
        core_load[c] += degin[n]
        core_cnt[c] += 1
    lo_mask = node_core[src] < 4
    deg_lo = np.bincount(dst[lo_mask], minlength=N).astype(np.int64)
    deg_hi = degin - deg_lo
    # per core, nodes -> groups balancing (lo,hi) in-degree, then slot
    perm = np.empty(N, dtype=np.int64)
    gcap = np.full(NGROUPS, GW, dtype=np.int64)
    gcap[NGROUPS - 1] = (NB - (NGROUPS - 1) * GROUP_BLOCKS) * P
    for c in range(NCORES):
        nodes = np.where(node_core == c)[0]
        nd = nodes[np.argsort(-(deg_lo[nodes] + deg_hi[nodes]), kind="stable")]
        glo = np.zeros(NGROUPS, dtype=np.int64)
        ghi = np.zeros(NGROUPS, dtype=np.int64)
        gcnt = np.zeros(NGROUPS, dtype=np.int64)
        slot = np.zeros(NGROUPS, dtype=np.int64)
        for n in nd:
            g = np.argmin(np.where(gcnt < gcap, glo + ghi,
                                   np.iinfo(np.int64).max))
            perm[n] = c * NODES_PER + g * GW + slot[g]
            glo[g] += deg_lo[n]
            ghi[g] += deg_hi[n]
            gcnt[g] += 1
            slot[g] += 1
    return perm


def _build_schedule(srcrow, dstrow):
    """Packed shape-uniform SPMD edge schedule (no self-loops in stream)."""
    core = dstrow // NODES_PER
    loc = dstrow % NODES_PER
    grp = loc // GW
    gdl = loc % GW
    bkt = (srcrow >= R4).astype(np.int64)

    # order: core, group, bucket, block-in-group (gdl)
    order = np.lexsort((gdl, bkt, grp, core))
    s_o = srcrow[order]
    g_o = gdl[order]
    key = ((core * NGROUPS + grp) * 2 + bkt)[order]
    ncell = NCORES * NGROUPS * 2
    cuts = np.searchsorted(key, np.arange(ncell + 1))
    cnt = np.diff(cuts).reshape(NCORES, NGROUPS, 2)

    T = np.zeros((NGROUPS, 2), dtype=np.int64)
    for g in range(NGROUPS):
        for k in (0, 1):
            T[g, k] = (cnt[:, g, k].max() + P - 1) // P

    group_info = []
    toff = 0
    for g in range(NGROUPS):
        group_info.append((toff, int(T[g, 0]), int(T[g, 1])))
        toff += int(T[g, 0] + T[g, 1])
    ntiles = toff

    hidx = np.zeros((NCORES, ntiles * P), dtype=np.int16)
    dloc = np.full((NCORES, ntiles * P), -1.0, dtype=np.float32)
    # block sets per (group, tile-in-group): union over cores
    nblocks_g = [min(GROUP_BLOCKS, NB - g * GROUP_BLOCKS)
                 for g in range(NGROUPS)]
    touch = [np.zeros((T[g, 0] + T[g, 1], nblocks_g[g]), dtype=bool)
             for g in range(NGROUPS)]
    for c in range(NCORES):
        for g in range(NGROUPS):
            toff_g, tlo, thi = group_info[g]
            for k in (0, 1):
                cell = (c * NGROUPS + g) * 2 + k
                s = s_o[cuts[cell]:cuts[cell + 1]]
                d = g_o[cuts[cell]:cuts[cell + 1]]
                n = len(s)
                pos = (toff_g + (0 if k == 0 else tlo)) * P
                hidx[c, pos:pos + n] = (s - k * R4).astype(np.int16)
                dloc[c, pos:pos + n] = d.astype(np.float32)
                jt0 = 0 if k == 0 else tlo
                if n:
                    touch[g][jt0 + np.arange(n) // P, d // P] = True

    # per group: ordered op list [(tile_in_group, brel)] + start/stop flags
    ops_per_group = []
    for g in range(NGROUPS):
        ops = []
        tg = int(T[g, 0] + T[g, 1])
        for jt in range(tg):
            for brel in range(nblocks_g[g]):
                if touch[g][jt, brel]:
                    ops.append((jt, brel))
        first = {}
        last = {}
        for i, (jt, brel) in enumerate(ops):
            if brel not in first:
                first[brel] = i
            last[brel] = i
        flag_ops = [(jt, brel, i == first[brel], i == last[brel])
                    for i, (jt, brel) in enumerate(ops)]
        # blocks with no edges anywhere: mark for memset
        empty = [brel for brel in range(nblocks_g[g]) if brel not in first]
        ops_per_group.append((flag_ops, empty))

    def wrap16(a):
        m = a.reshape(-1, 16).T
        return np.tile(m, (8, 1)).copy()

    return {
        "hidx": np.stack([wrap16(hidx[c]) for c in range(NCORES)]),
        "dloc": np.stack([dloc[c].reshape(ntiles, P).T.copy()
                          for c in range(NCORES)]),   # [128, ntiles]
        # per-edge group-local dst id, replicated on all 128 partitions
        "dloct": np.stack([np.tile(dloc[c].astype(npbf16), (P, 1))
                           for c in range(NCORES)]),  # [128, ntiles*128] bf16
        "group_info": group_info,
        "ops_per_group": ops_per_group,
        "nblocks_g": nblocks_g,
        "ntiles": ntiles,
    }


def _augment_weights(W, asrc, adst):
    """[W | W@Asrc | W@Adst] -> [IN_, HCl + 2H] float32."""
    IN_, HCl = W.shape
    H = asrc.shape[0]
    C = HCl // H
    A_s = np.zeros((HCl, H), np.float32)
    A_d = np.zeros((HCl, H), np.float32)
    for hd in range(H):
        A_s[hd * C:(hd + 1) * C, hd] = asrc[hd]
        A_d[hd * C:(hd + 1) * C, hd] = adst[hd]
    return np.concatenate([W, W @ A_s, W @ A_d], axis=1)


# ----------------------------------------------------------------------------
# device program
# ----------------------------------------------------------------------------

def _build_program(sched, time_reps=1,
                   stages=("d1", "e1", "d2", "e2", "d3", "e3", "pool"),
                   sim_single=False):
    nc = bacc.Bacc("TRN2", target_bir_lowering=False, debug=False,
                   enable_asserts=False,
                   num_devices=1 if sim_single else NCORES,
                   num_swdge_queues=2)
    ntiles = sched["ntiles"]
    group_info = sched["group_info"]
    ops_per_group = sched["ops_per_group"]
    nblocks_g = sched["nblocks_g"]
    tgmax = max(tlo + thi for _, tlo, thi in group_info)

    # ---- kernel I/O ----
    t_xT = nc.dram_tensor("xT", [P, NODES_PER], bf16, kind="ExternalInput")
    t_w1 = nc.dram_tensor("W1a", [P, 264], bf16, kind="ExternalInput")
    t_w2 = nc.dram_tensor("W2a", [P, 2, 264], bf16, kind="ExternalInput")
    t_w3 = nc.dram_tensor("W3a", [P, 2, 66], bf16, kind="ExternalInput")
    t_b1 = nc.dram_tensor("BIAS1", [P, HC], f32, kind="ExternalInput")
    t_b2 = nc.dram_tensor("BIAS2", [P, HC], f32, kind="ExternalInput")
    t_b3 = nc.dram_tensor("BIAS3", [P, HID], f32, kind="ExternalInput")
    t_iota = nc.dram_tensor("IOTA4", [P, TBATCH * GW], f32,
                            kind="ExternalInput")
    t_ident = nc.dram_tensor("IDENT", [P, P], f32, kind="ExternalInput")
    t_hidx = nc.dram_tensor("HIDX", [P, ntiles * 8], i16, kind="ExternalInput")
    t_dloc = nc.dram_tensor("DLOC", [P, ntiles], f32, kind="ExternalInput")
    t_dloct = nc.dram_tensor("DLOCT", [P, ntiles * P], bf16,
                             kind="ExternalInput")
    t_iotap = nc.dram_tensor("IOTAP", [P, GW], bf16, kind="ExternalInput")
    t_bat = nc.dram_tensor("BATCH", [P, NB], f32, kind="ExternalInput")
    t_icnt = nc.dram_tensor("INVCNT", [NG, 1], f32, kind="ExternalInput")
    t_linw = nc.dram_tensor("LINW", [P, OUT], f32r, kind="ExternalInput")
    t_linb = nc.dram_tensor("LINB", [NG, OUT], f32, kind="ExternalInput")
    t_out = nc.dram_tensor("out", [NG, OUT], f32, kind="ExternalOutput")
    t_dbg1 = t_dbg2 = None
    if "pool" not in stages:
        t_dbg1 = nc.dram_tensor("dbg1", [P, 264], f32, kind="ExternalOutput")
        t_dbg2 = nc.dram_tensor("dbg2", [P, 2 * P], f32, kind="ExternalOutput")

    layers = [
        dict(ew=EW12, hc=HC, h=HEADS, nchunk=1, wcols=264, rw=260),
        dict(ew=EW12, hc=HC, h=HEADS, nchunk=2, wcols=264, rw=260),
        dict(ew=EW3, hc=HID, h=1, nchunk=2, wcols=66, rw=66),
    ]

    with tile.TileContext(nc) as tc:
        with tc.tile_pool(name="const", bufs=1) as cpool, \
             tc.tile_pool(name="xT", bufs=1) as xpool, \
             tc.tile_pool(name="gat", bufs=2) as gpool, \
             tc.tile_pool(name="work", bufs=4) as wpool, \
             tc.tile_pool(name="fin", bufs=2) as fpool, \
             tc.tile_pool(name="psacc", bufs=4, space="PSUM") as psacc, \
             tc.tile_pool(name="psmisc", bufs=2, space="PSUM") as psmisc, \
             tc.tile_pool(name="pspool", bufs=1, space="PSUM") as pspool, \
             tc.tile_pool(name="dram", bufs=1, space="DRAM") as dpool:

            # ---- consts into SBUF ----
            iota4 = cpool.tile([P, TBATCH, GW], f32)
            nc.sync.dma_start(iota4[:], t_iota.ap().rearrange(
                "p (t i) -> p t i", t=TBATCH))
            iotap = cpool.tile([P, GROUP_BLOCKS, P], bf16)
            nc.sync.dma_start(iotap[:], t_iotap.ap().rearrange(
                "p (t i) -> p t i", t=GROUP_BLOCKS))
            ident = cpool.tile([P, P], f32)
            nc.sync.dma_start(ident[:], t_ident.ap())
            biases = []
            for tb, w in ((t_b1, HC), (t_b2, HC), (t_b3, HID)):
                bt = cpool.tile([P, w], f32, tag=f"bias{len(biases)}",
                                name=f"bias{len(biases)}")
                nc.sync.dma_start(bt[:], tb.ap())
                biases.append(bt)
            w1t = cpool.tile([P, 1, 264], bf16, tag="w1")
            nc.sync.dma_start(w1t[:, 0, :], t_w1.ap())
            w2t = cpool.tile([P, 2, 264], bf16, tag="w2")
            nc.sync.dma_start(w2t[:], t_w2.ap())
            w3t = cpool.tile([P, 2, 66], bf16, tag="w3")
            nc.sync.dma_start(w3t[:], t_w3.ap())
            w_sb = [w1t, w2t, w3t]
            batc = cpool.tile([P, NB], f32)
            nc.sync.dma_start(batc[:], t_bat.ap())
            icnt = cpool.tile([NG, 1], f32)
            nc.sync.dma_start(icnt[:], t_icnt.ap())
            linw = cpool.tile([P, OUT], f32r)
            nc.sync.dma_start(linw[:], t_linw.ap())
            linb = cpool.tile([NG, OUT], f32)
            nc.sync.dma_start(linb[:], t_linb.ap())

            # persistent transposed activations (bf16) + self-loop side data
            xT = xpool.tile([P, 2, NODES_PER], bf16)
            nc.sync.dma_start(xT[:, 0, :], t_xT.ap())
            # own-block dense outputs [h | alsrc | aldst] for the self-loop
            # path and the aldst one-hot matmuls
            hall = xpool.tile([P, NB, 264], bf16)

            # per-layer DRAM scratch
            tslice = [dpool.tile([NODES_PER, EW12], bf16, tag="ts0", name="ts0"),
                      dpool.tile([NODES_PER, EW12], bf16, tag="ts1", name="ts1"),
                      dpool.tile([NODES_PER, EW3], bf16, tag="ts2", name="ts2")]
            tfull = [dpool.tile([NTOT, EW12], bf16, tag="tf0", name="tf0",
                                addr_space="Shared"),
                     dpool.tile([NTOT, EW12], bf16, tag="tf1", name="tf1",
                                addr_space="Shared"),
                     dpool.tile([NTOT, EW3], bf16, tag="tf2", name="tf2",
                                addr_space="Shared")]
            pool_in = dpool.tile([NG, HID], f32, tag="pin")
            pool_out = dpool.tile([NG, HID], f32, tag="pout")

            pool_ps = pspool.tile([NG, HID], f32, space="PSUM")

            def dense_phase(L):
                lay = layers[L]
                hc, h, nchunk, wcols = lay["hc"], lay["h"], lay["nchunk"], lay["wcols"]
                for m in range(NB):
                    ps = psmisc.tile([P, wcols], f32, space="PSUM", tag="ms",
                                     name="dps")
                    for c in range(nchunk):
                        nc.tensor.matmul(
                            ps[:], xT[:, c, m * P:(m + 1) * P], w_sb[L][:, c, :],
                            start=(c == 0), stop=(c == nchunk - 1))
                    nc.vector.tensor_copy(hall[:, m, 0:hc + 2 * h],
                                          ps[:, 0:hc + 2 * h])
                    nc.sync.dma_start(
                        tslice[L][m * P:(m + 1) * P, 0:hc + h],
                        hall[:, m, 0:hc + h])

            def edge_phase(L):
                lay = layers[L]
                ew, hc, h, rw = lay["ew"], lay["hc"], lay["h"], lay["rw"]
                for g in range(NGROUPS):
                    toff, tlo, thi = group_info[g]
                    flag_ops, empty = ops_per_group[g]
                    tg = tlo + thi
                    nbg = nblocks_g[g]
                    G = gpool.tile([P, tgmax, ew], bf16, tag="G")
                    hix = gpool.tile([P, tgmax * 8], i16, tag="hix")
                    dlc = gpool.tile([P, tgmax], f32, tag="dlc")
                    dlt = gpool.tile([P, tgmax, P], bf16, tag="dlt")
                    o16 = toff * 8
                    nc.sync.dma_start(hix[:, 0:tg * 8],
                                      t_hidx.ap()[:, o16:o16 + tg * 8])
                    nc.sync.dma_start(dlc[:, 0:tg], t_dloc.ap()[:, toff:toff + tg])
                    nc.sync.dma_start(
                        dlt[:, 0:tg, :],
                        t_dloct.ap()[:, toff * P:(toff + tg) * P].rearrange(
                            "p (t i) -> p t i", t=tg))
                    if tlo:
                        nc.gpsimd.dma_gather(
                            G[:, 0:tlo, :], tfull[L][0:R4, :],
                            hix[:, 0:tlo * 8], num_idxs=tlo * P,
                            num_idxs_reg=tlo * P, elem_size=ew, elem_step=ew,
                            single_packet=False, queue_num=0)
                    if thi:
                        nc.gpsimd.dma_gather(
                            G[:, tlo:tg, :], tfull[L][R4:NTOT, :],
                            hix[:, tlo * 8:tg * 8], num_idxs=thi * P,
                            num_idxs_reg=thi * P, elem_size=ew, elem_step=ew,
                            single_packet=False, queue_num=1)

                    if EDGE_MODE == "gather":
                        junk = wpool.tile([P, 8], f32, tag="junk")
                        nc.vector.tensor_tensor(
                            junk[:, 0:4], G[:, 0, 0:4], dlt[:, 0, 0:4],
                            mybir.AluOpType.add)
                        continue

                    accs = {brel: psacc.tile([P, rw], f32, space="PSUM",
                                             tag="acc", name=f"acc{brel}")
                            for brel in range(nbg)}
                    for brel in empty:
                        nc.vector.memset(accs[brel][:], 0.0)

                    # ops grouped by tile batch
                    j = 0
                    oi = 0
                    while j < tg:
                        w = min(TBATCH, tg - j)
                        S = wpool.tile([P, TBATCH, GW], bf16, tag="S")
                        nc.vector.tensor_tensor(
                            S[:, 0:w, :],
                            dlc[:, j:j + w].unsqueeze(-1).to_broadcast(
                                [P, w, GW]),
                            iota4[:, 0:w, :], mybir.AluOpType.is_equal)
                        # per-edge aldst via one-hot matmuls against the
                        # SBUF-resident aldsb (no DMA gather)
                        S2 = wpool.tile([P, TBATCH, GROUP_BLOCKS, P], bf16,
                                        tag="S2")
                        for kc in range(nbg):
                            nc.vector.tensor_tensor(
                                S2[:, 0:w, kc, :], dlt[:, j:j + w, :],
                                iotap[:, kc, :].unsqueeze(1).to_broadcast(
                                    [P, w, P]),
                                mybir.AluOpType.is_equal)
                        ald_ps = psmisc.tile([P, TBATCH, HEADS], f32,
                                             space="PSUM", tag="ms",
                                             name="aldps")
                        for q in range(w):
                            for kc in range(nbg):
                                nc.tensor.matmul(
                                    ald_ps[:, q, 0:h], S2[:, q, kc, :],
                                    hall[:, g * GROUP_BLOCKS + kc,
                                         hc + h:hc + 2 * h],
                                    start=(kc == 0), stop=(kc == nbg - 1),
                                    skip_group_check=True)
                        ald_sb = wpool.tile([P, TBATCH, HEADS], bf16,
                                            tag="aldsbt")
                        nc.vector.tensor_copy(ald_sb[:, 0:w, 0:h],
                                              ald_ps[:, 0:w, 0:h])
                        et = wpool.tile([P, TBATCH, h], f32, tag="et")
                        nc.vector.tensor_tensor(
                            et[:, 0:w, :],
                            G[:, j:j + w, hc:hc + h],
                            ald_sb[:, 0:w, 0:h], mybir.AluOpType.add)
                        lr = wpool.tile([P, TBATCH, h], f32, tag="lr")
                        nc.vector.scalar_tensor_tensor(
                            lr[:, 0:w, :], et[:, 0:w, :], 0.2, et[:, 0:w, :],
                            mybir.AluOpType.mult, mybir.AluOpType.max)
                        ex = wpool.tile([P, TBATCH, h], f32, tag="ex")
                        nc.scalar.activation(ex[:, 0:w, :], lr[:, 0:w, :],
                                             mybir.ActivationFunctionType.Exp)
                        exb = wpool.tile([P, TBATCH, h], bf16, tag="exb")
                        nc.vector.tensor_copy(exb[:, 0:w, :], ex[:, 0:w, :])
                        R = wpool.tile([P, TBATCH, rw], bf16, tag="R")
                        nc.vector.tensor_tensor(
                            R[:, 0:w, 0:hc].rearrange(
                                "p t (hh c) -> p t hh c", hh=h),
                            G[:, j:j + w, 0:hc].rearrange(
                                "p t (hh c) -> p t hh c", hh=h),
                            exb[:, 0:w, :].unsqueeze(-1).to_broadcast(
                                [P, w, h, HID]),
                            mybir.AluOpType.mult)
                        # vector (not scalar.activation Copy): keeps the
                        # Activation engine on Exp only — no act-table reloads
                        if rw == hc + h:
                            nc.vector.tensor_copy(R[:, 0:w, hc:hc + h],
                                                  ex[:, 0:w, :])
                        else:
                            # odd tail (L3): fill cols hc:rw with ex so the
                            # matmul never reads uninitialized SBUF
                            nc.vector.tensor_copy(
                                R[:, 0:w, hc:rw],
                                ex[:, 0:w, 0:1].to_broadcast([P, w, rw - hc]))
                        while oi < len(flag_ops) and flag_ops[oi][0] < j + w:
                            jt, brel, st, sp = flag_ops[oi]
                            nc.tensor.matmul(
                                accs[brel][:], S[:, jt - j, brel * P:(brel + 1) * P],
                                R[:, jt - j, 0:rw],
                                start=st, stop=sp, skip_group_check=True)
                            oi += 1
                        j += w

                    if EDGE_MODE == "compute":
                        for brel in range(nbg):
                            junk2 = wpool.tile([P, 8], f32, tag="junk2")
                            nc.vector.tensor_copy(junk2[:, 0:4],
                                                  accs[brel][:, 0:4])
                        continue

                    for brel in range(nbg):
                        b = g * GROUP_BLOCKS + brel
                        acc = accs[brel]
                        # self-loop attention term (all SBUF)
                        ets = wpool.tile([P, h], f32, tag="ets")
                        nc.vector.tensor_tensor(ets[:], hall[:, b, hc:hc + h],
                                                hall[:, b, hc + h:hc + 2 * h],
                                                mybir.AluOpType.add)
                        lrs = wpool.tile([P, h], f32, tag="lrs")
                        nc.vector.scalar_tensor_tensor(
                            lrs[:], ets[:], 0.2, ets[:],
                            mybir.AluOpType.mult, mybir.AluOpType.max)
                        exs = wpool.tile([P, h], f32, tag="exs")
                        nc.scalar.activation(exs[:], lrs[:],
                                             mybir.ActivationFunctionType.Exp)
                        exsb = wpool.tile([P, h], bf16, tag="exsb")
                        nc.vector.tensor_copy(exsb[:], exs[:])
                        den = wpool.tile([P, h], f32, tag="den")
                        nc.vector.tensor_tensor(den[:], acc[:, hc:hc + h],
                                                exs[:], mybir.AluOpType.add)
                        rec = wpool.tile([P, h], f32, tag="rec")
                        nc.vector.reciprocal(rec[:], den[:])
                        selfc = fpool.tile([P, HC], f32, tag="selfc")
                        nc.vector.tensor_tensor(
                            selfc[:, 0:hc].rearrange("p (hh c) -> p hh c", hh=h),
                            hall[:, b, 0:hc].rearrange("p (hh c) -> p hh c", hh=h),
                            exsb[:].unsqueeze(-1).to_broadcast([P, h, HID]),
                            mybir.AluOpType.mult)
                        num = fpool.tile([P, HC], f32, tag="num")
                        nc.vector.tensor_tensor(num[:, 0:hc], acc[:, 0:hc],
                                                selfc[:, 0:hc],
                                                mybir.AluOpType.add)
                        xb = fpool.tile([P, HC], f32, tag="xb")
                        nc.vector.tensor_tensor(
                            xb[:, 0:hc].rearrange("p (hh c) -> p hh c", hh=h),
                            num[:, 0:hc].rearrange("p (hh c) -> p hh c", hh=h),
                            rec[:].unsqueeze(-1).to_broadcast([P, h, HID]),
                            mybir.AluOpType.mult)
                        nc.vector.tensor_tensor(xb[:, 0:hc], xb[:, 0:hc],
                                                biases[L][:, 0:hc],
                                                mybir.AluOpType.add)
                        # ELU = max(x,0) + exp(min(x,0)) - 1
                        xmin = fpool.tile([P, HC], f32, tag="xmin")
                        nc.vector.tensor_scalar_min(xmin[:, 0:hc], xb[:, 0:hc],
                                                    0.0)
                        em = fpool.tile([P, HC], f32, tag="em")
                        nc.scalar.activation(em[:, 0:hc], xmin[:, 0:hc],
                                             mybir.ActivationFunctionType.Exp)
                        xmax = fpool.tile([P, HC], f32, tag="xmax")
                        nc.vector.tensor_scalar_max(xmax[:, 0:hc], xb[:, 0:hc],
                                                    0.0)
                        if L < 2:
                            x2b = fpool.tile([P, HC], f32, tag="x2b")
                            nc.vector.scalar_tensor_tensor(
                                x2b[:], em[:], -1.0, xmax[:],
                                mybir.AluOpType.add, mybir.AluOpType.add)
                            for cchunk in range(2):
                                pt = psmisc.tile([P, P], f32, space="PSUM",
                                                 tag="ms", name="pt")
                                nc.tensor.transpose(
                                    pt[:], x2b[:, cchunk * P:(cchunk + 1) * P],
                                    ident[:])
                                nc.vector.tensor_copy(
                                    xT[:, cchunk, b * P:(b + 1) * P], pt[:])
                        else:
                            x4 = fpool.tile([P, HID], bf16, tag="x4")
                            nc.vector.scalar_tensor_tensor(
                                x4[:], em[:, 0:HID], -1.0, xmax[:, 0:HID],
                                mybir.AluOpType.add, mybir.AluOpType.add)
                            bsel = fpool.tile([P, NG], bf16, tag="bsel")
                            nc.vector.tensor_tensor(
                                bsel[:],
                                batc[:, b:b + 1].to_broadcast([P, NG]),
                                iota4[:, 0, 0:NG], mybir.AluOpType.is_equal)
                            nc.tensor.matmul(
                                pool_ps[:], bsel[:], x4[:],
                                start=(b == 0), stop=(b == NB - 1),
                                skip_group_check=True)

            reps = max(1, int(time_reps))

            def repeat(fn):
                if reps == 1:
                    fn()
                else:
                    with tc.For_i(0, reps, 1):
                        fn()

            for L in range(3):
                if f"d{L + 1}" in stages:
                    repeat(lambda L=L: dense_phase(L))
                    if sim_single:
                        for k in range(NCORES):
                            nc.sync.dma_start(
                                tfull[L][k * NODES_PER:(k + 1) * NODES_PER, :],
                                tslice[L][:, :])
                    else:
                        nc.gpsimd.collective_compute(
                            "AllGather", mybir.AluOpType.bypass,
                            replica_groups=[list(range(NCORES))],
                            ins=[tslice[L].opt()], outs=[tfull[L].opt()])
                if f"e{L + 1}" in stages:
                    repeat(lambda L=L: edge_phase(L))

            if "pool" not in stages:
                dts = wpool.tile([P, 264], f32, tag="dts")
                nc.vector.memset(dts[:], 0.0)
                nc.sync.dma_start(t_dbg1.ap(), dts[:])
                dxt = wpool.tile([P, 2 * P], f32, tag="dxt")
                nc.vector.tensor_copy(dxt[:, 0:P], xT[:, 0, 0:P])
                nc.vector.tensor_copy(dxt[:, P:2 * P], xT[:, 1, 0:P])
                nc.sync.dma_start(t_dbg2.ap(), dxt[:])
                fin0 = wpool.tile([NG, OUT], f32, tag="finout")
                nc.vector.memset(fin0[:], 0.0)
                nc.sync.dma_start(t_out.ap(), fin0[:])
            else:
                pp = wpool.tile([NG, HID], f32, tag="pp")
                nc.vector.tensor_copy(pp[:], pool_ps[:])
                if sim_single:
                    nc.sync.dma_start(pool_in[:], pp[:])
                    nc.sync.dma_start(pool_out[:], pool_in[:])
                else:
                    nc.sync.dma_start(pool_in[:], pp[:])
                    nc.gpsimd.collective_compute(
                        "AllReduce", mybir.AluOpType.add,
                        replica_groups=[list(range(NCORES))],
                        ins=[pool_in.opt()], outs=[pool_out.opt()])
                pooled = wpool.tile([NG, HID], f32, tag="pooled")
                nc.sync.dma_start(pooled[:], pool_out[:])
                pscal = wpool.tile([NG, HID], f32, tag="pscal")
                nc.scalar.activation(pscal[:], pooled[:],
                                     mybir.ActivationFunctionType.Copy,
                                     scale=icnt[:])
                ptp = psmisc.tile([NG, NG], f32, space="PSUM", tag="ms",
                                  name="ptp")
                nc.tensor.transpose(ptp[:], pscal[:, 0:NG], ident[0:NG, 0:NG])
                zconst = wpool.tile([P, NG], f32, tag="zconst")
                nc.vector.memset(zconst[:], 0.0)
                pT = wpool.tile([P, NG], f32r, tag="pT")
                nc.vector.tensor_copy(pT[:], zconst[:])
                nc.vector.tensor_copy(pT[0:NG, :], ptp[:])
                ops = psmisc.tile([NG, OUT], f32, space="PSUM", tag="ms",
                                  name="ops")
                nc.tensor.matmul(ops[:], pT[:], linw[:], start=True, stop=True)
                fin = wpool.tile([NG, OUT], f32, tag="finout")
                nc.vector.tensor_tensor(fin[:], ops[:], linb[:],
                                        mybir.AluOpType.add)
                nc.sync.dma_start(t_out.ap(), fin[:])

    nc.compile()
    return nc


# ----------------------------------------------------------------------------
# host orchestration
# ----------------------------------------------------------------------------

def _prepare(inputs):
    x = np.asarray(inputs["x"], dtype=np.float32)
    ei = np.asarray(inputs["edge_index"])
    batch = np.asarray(inputs["batch"])
    src = ei[0].astype(np.int64)
    dst = ei[1].astype(np.int64)

    perm = _assign_nodes(src, dst)
    srcrow = perm[src]
    dstrow = perm[dst]
    sched = _build_schedule(srcrow, dstrow)

    row_node = np.full(NTOT, -1, dtype=np.int64)
    row_node[perm] = np.arange(N)

    w1a = _augment_weights(np.asarray(inputs["W1"], np.float32),
                           np.asarray(inputs["asrc1"], np.float32),
                           np.asarray(inputs["adst1"], np.float32))
    w2a = _augment_weights(np.asarray(inputs["W2"], np.float32),
                           np.asarray(inputs["asrc2"], np.float32),
                           np.asarray(inputs["adst2"], np.float32))
    w3a = _augment_weights(np.asarray(inputs["W3"], np.float32),
                           np.asarray(inputs["asrc3"], np.float32),
                           np.asarray(inputs["adst3"], np.float32))

    iota4 = np.tile(np.arange(GW, dtype=np.float32), (P, TBATCH)).reshape(
        P, TBATCH * GW)
    iotap = np.zeros((P, GW), np.float32)
    for kc in range(GROUP_BLOCKS):
        iotap[:, kc * P:(kc + 1) * P] = (kc * P + np.arange(P))[:, None]
    iotap = iotap.astype(npbf16)
    ident = np.eye(P, dtype=np.float32)

    cnts = np.bincount(batch, minlength=NG).astype(np.float32)
    invcnt = (1.0 / np.maximum(cnts, 1.0)).reshape(NG, 1)
    linb = np.tile(np.asarray(inputs["linb"], np.float32), (NG, 1))

    in_maps = []
    for c in range(NCORES):
        rows = row_node[c * NODES_PER:(c + 1) * NODES_PER]
        xT = np.zeros((P, NODES_PER), npbf16)
        valid = rows >= 0
        xT[:, valid] = x[rows[valid]].T.astype(npbf16)
        batc = np.full((NB, P), -1.0, np.float32)
        bflat = batc.reshape(-1)
        bflat[valid] = batch[rows[valid]].astype(np.float32)
        in_maps.append({
            "xT": xT,
            "W1a": w1a.astype(npbf16),
            "W2a": np.stack([w2a[0:P], w2a[P:2 * P]], axis=1).astype(npbf16),
            "W3a": np.stack([w3a[0:P], w3a[P:2 * P]], axis=1).astype(npbf16),
            "BIAS1": np.tile(np.asarray(inputs["b1"], np.float32), (P, 1)),
            "BIAS2": np.tile(np.asarray(inputs["b2"], np.float32), (P, 1)),
            "BIAS3": np.tile(np.asarray(inputs["b3"], np.float32), (P, 1)),
            "IOTA4": iota4,
            "IOTAP": iotap,
            "IDENT": ident,
            "HIDX": sched["hidx"][c],
            "DLOC": sched["dloc"][c],
            "DLOCT": sched["dloct"][c],
            "BATCH": batc.T.copy(),
            "INVCNT": invcnt,
            "LINW": np.concatenate(
                [np.asarray(inputs["linW"], np.float32),
                 np.zeros((P - HID, OUT), np.float32)], axis=0),
            "LINB": linb,
        })
    return sched, in_maps


def kernel(**inputs):
    sched, in_maps = _prepare(inputs)
    nc = _build_program(sched, time_reps=1)
    res = bass_utils.run_bass_kernel_spmd(nc, in_maps, core_ids=list(range(NCORES)))
    return res.results[0]["out"].astype(np.float32)
